# revision 1
# baseline (speedup 1.0000x reference)
"""Trainium2 kernel for nn_DecouplingFlowLayer.

Reference computation (per (batch, stock) row):
  - channel 0 of x undergoes a Haar DWT + linear upsample back to S
    (low band Xl, high band Xh)
  - Xl (resp. Xh) is concatenated with channels 1..F-1 and projected by
    Wg (resp. Wh):  out = [others, X*] @ W.T + b

Host does the (tiny, ~1MB) DWT/interp exactly as the reference, then
packs a 364-feature tensor x2 = [Xl, Xh, ch1..ch361, 1.0] per token
(the ones column folds the bias in), already bf16 and feature-major, so
the device work is a pure double GEMM
    out[t, 0:128]   = x2[t] @ Wg2.T
    out[t, 128:256] = x2[t] @ Wh2.T
sharded over 8 NeuronCores by stock (32 stocks/core, 32768 tokens/core).

Device kernel (per core, bf16 compute / fp32 PSUM accumulate):
  - input DRAM layout [slab, chunk, 128, SLAB]: feature-major, one
    contiguous full-rate DMA per 4096-token slab. K=364 is covered by 3
    row-chunks of 128 (chunk 2 overlaps chunk 1 in rows 236..255; the
    duplicated weight rows are zeroed so the accumulation is exact).
  - per slab, 2 output halves x 2 PSUM waves x (3 K-chunks x 4 groups)
    of [128x128] x [128x512] matmuls accumulate into PSUM banks;
    ScalarE/VectorE copy+cast fp32 PSUM -> bf16 SBUF.
  - output DRAM layout [slab, 128, 2, SLAB] (d-major, bf16): one
    contiguous DMA per slab with 16KB/partition descriptors. The host
    de-transposes/casts while assembling the final fp32 arrays.

This keeps the kernel at the HBM roofline: 25.2 MB in + 16.8 MB out
per core at ~358 GB/s ~= 120 us, with PE (~92 us) and DVE/ACT (~40 us
each) hidden underneath.
"""

import os
import numpy as np
import ml_dtypes

import concourse.bacc as bacc
import concourse.mybir as mybir
import concourse.tile as tile
from concourse.bass_utils import run_bass_kernel_spmd

B, S, N, F = 2, 512, 256, 362
D = 128
NCORES = 8
NSH = N // NCORES          # 32 stocks per core
T = B * S * NSH            # 32768 tokens per core
K = F + 2                  # Xl, Xh, ch1..ch361, ones  -> 364
CHUNK_OFF = (0, 128, 236)  # chunk 2 overlaps rows 236..255 (those wt rows zeroed)
GROUP = 512                # matmul moving-dim granularity (PSUM bank = 512 fp32)
SLAB = 4096                # tokens per DMA slab
NSLABS = T // SLAB         # 8
QPS = SLAB // GROUP        # groups per slab = 8
QBLK = 4                   # PSUM banks per accumulation wave (c-outer within)

BF16 = mybir.dt.bfloat16
F32 = mybir.dt.float32
OUT_BF16 = os.environ.get("KRN_OUT_F32", "0") != "1"
OUT_DT = BF16 if OUT_BF16 else F32
OUT_NP = ml_dtypes.bfloat16 if OUT_BF16 else np.float32

_NC_CACHE = {}
TRACE = False
LAST_RESULT = None


def _build(repeat=1):
    key = (OUT_BF16, repeat)
    if key in _NC_CACHE:
        return _NC_CACHE[key]
    nc = bacc.Bacc(None, target_bir_lowering=False)
    x2d = nc.dram_tensor("x2", [NSLABS, 3, 128, SLAB], BF16, kind="ExternalInput")
    w2d = nc.dram_tensor("w2", [3, 128, 256], BF16, kind="ExternalInput")
    outd = nc.dram_tensor("out", [NSLABS, 128, 2, SLAB], OUT_DT, kind="ExternalOutput")

    with tile.TileContext(nc) as tc:
        with (
            tc.tile_pool(name="cpool", bufs=1) as cpool,
            tc.tile_pool(name="xpool", bufs=4) as xpool,
            tc.tile_pool(name="spool", bufs=4) as spool,
            tc.tile_pool(name="psA", bufs=8, space="PSUM") as psA,
        ):
            wt = cpool.tile([128, 3, 256], BF16)
            nc.sync.dma_start(wt[:, :, :], w2d[:, :, :].rearrange("c p d -> p c d"))

            for rep in range(repeat):
                for s in range(NSLABS):
                    xt = xpool.tile([128, 3, SLAB], BF16, tag="xt")
                    nc.sync.dma_start(
                        xt[:, :, :], x2d[s, :, :, :].rearrange("c p t -> p c t")
                    )
                    so = spool.tile([128, 2, SLAB], OUT_DT, tag="so")
                    for h in range(2):
                        for qb in range(QPS // QBLK):
                            accs = [
                                psA.tile([128, GROUP], F32, tag="acc",
                                         name=f"acc{rep}_{s}_{h}_{qb}_{i}")
                                for i in range(QBLK)
                            ]
                            # c outer / q inner: one LDWEIGHTS per c feeds
                            # QBLK moving streams
                            for c in range(3):
                                for i in range(QBLK):
                                    q = qb * QBLK + i
                                    nc.tensor.matmul(
                                        accs[i][:, :],
                                        wt[:, c, h * 128 : (h + 1) * 128],
                                        xt[:, c, q * GROUP : (q + 1) * GROUP],
                                        start=(c == 0),
                                        stop=(c == 2),
                                    )
                            for i in range(QBLK):
                                q = qb * QBLK + i
                                dst = so[:, h, q * GROUP : (q + 1) * GROUP]
                                if (h + i) % 2 == 0:
                                    nc.scalar.copy(dst, accs[i][:, :])
                                else:
                                    nc.vector.tensor_copy(dst, accs[i][:, :])
                    nc.scalar.dma_start(outd[s, :, :, :], so[:, :, :])
    nc.finalize()
    _NC_CACHE[key] = nc
    return nc


def _haar_interp_host(x):
    """Exact fp32 replica of the reference DWT+interp, on [B, S, N] ch0."""
    r = np.ascontiguousarray(np.transpose(x[:, :, :, 0], (0, 2, 1)))  # [B, N, S]
    inv = np.float32(1.0 / np.sqrt(2.0))
    pairs = r.reshape(B, N, S // 2, 2)
    cA = (pairs[..., 0] + pairs[..., 1]) * inv
    cD = (pairs[..., 0] - pairs[..., 1]) * inv
    L = S // 2
    src = np.maximum((np.arange(S, dtype=np.float32) + 0.5) * (L / S) - 0.5, 0.0)
    i0 = np.floor(src).astype(np.int32)
    i1 = np.minimum(i0 + 1, L - 1)
    w = (src - i0.astype(np.float32)).astype(np.float32)

    def interp(c):
        return c[..., i0] * (np.float32(1.0) - w) + c[..., i1] * w  # [B, N, S]

    Xl = np.transpose(interp(cA), (0, 2, 1))  # [B, S, N]
    Xh = np.transpose(interp(cD), (0, 2, 1))
    return Xl, Xh


def _build_w2(Wg_w, Wg_b, Wh_w, Wh_b):
    W2T = np.zeros((K, 256), dtype=np.float32)
    W2T[0, :128] = Wg_w[:, F - 1]
    W2T[1, 128:] = Wh_w[:, F - 1]
    W2T[2 : F + 1, :128] = Wg_w[:, : F - 1].T
    W2T[2 : F + 1, 128:] = Wh_w[:, : F - 1].T
    W2T[F + 1, :128] = Wg_b
    W2T[F + 1, 128:] = Wh_b
    w2 = np.zeros((3, 128, 256), dtype=np.float32)
    w2[0] = W2T[0:128]
    w2[1] = W2T[128:256]
    w2[2, 20:, :] = W2T[256:K]  # rows 236..255 of chunk 2 zeroed (overlap w/ chunk 1)
    return np.ascontiguousarray(w2.astype(ml_dtypes.bfloat16))


def _core_input(x, Xl, Xh, core):
    """Build the feature-major bf16 slab layout [NSLABS, 3, 128, SLAB]."""
    n0 = core * NSH
    xa = np.ascontiguousarray(x[:, :, n0 : n0 + NSH, 1:]).reshape(T, F - 1)
    full = np.empty((K, T), dtype=ml_dtypes.bfloat16)
    full[2 : F + 1, :] = xa.T
    full[0, :] = Xl[:, :, n0 : n0 + NSH].reshape(T)
    full[1, :] = Xh[:, :, n0 : n0 + NSH].reshape(T)
    full[F + 1, :] = 1.0
    x2t = np.empty((NSLABS, 3, 128, SLAB), dtype=ml_dtypes.bfloat16)
    for c in range(3):
        x2t[:, c, :, :] = (
            full[CHUNK_OFF[c] : CHUNK_OFF[c] + 128]
            .reshape(128, NSLABS, SLAB)
            .swapaxes(0, 1)
        )
    return x2t


def kernel(x, Wg_w, Wg_b, Wh_w, Wh_b):
    global LAST_RESULT
    x = np.asarray(x, dtype=np.float32)
    Xl, Xh = _haar_interp_host(x)
    w2 = _build_w2(
        np.asarray(Wg_w, np.float32), np.asarray(Wg_b, np.float32),
        np.asarray(Wh_w, np.float32), np.asarray(Wh_b, np.float32),
    )

    from concurrent.futures import ThreadPoolExecutor
    with ThreadPoolExecutor(max_workers=8) as ex:
        shards = list(ex.map(lambda c: _core_input(x, Xl, Xh, c), range(NCORES)))
    in_maps = [{"x2": sh, "w2": w2} for sh in shards]

    nc = _build()
    res = run_bass_kernel_spmd(nc, in_maps, core_ids=list(range(NCORES)), trace=TRACE)
    LAST_RESULT = res

    Xl_proj = np.empty((B, S, N, D), dtype=np.float32)
    Xh_proj = np.empty((B, S, N, D), dtype=np.float32)
    for c in range(NCORES):
        o = res.results[c]["out"]  # [NSLABS, 128, 2, SLAB]
        o = np.transpose(o, (0, 3, 2, 1)).astype(np.float32)  # [NSLABS, SLAB, 2, 128]
        o = o.reshape(B, S, NSH, 2, D)
        n0 = c * NSH
        Xl_proj[:, :, n0 : n0 + NSH, :] = o[..., 0, :]
        Xh_proj[:, :, n0 : n0 + NSH, :] = o[..., 1, :]
    return Xl_proj, Xh_proj



# revision 2
# speedup vs baseline: 1.2226x; 1.2226x over previous
"""Trainium2 kernel for nn_DecouplingFlowLayer.

Reference computation (per (batch, stock) row):
  - channel 0 of x undergoes a Haar DWT + linear upsample back to S
    (low band Xl, high band Xh)
  - Xl (resp. Xh) is concatenated with channels 1..F-1 and projected by
    Wg (resp. Wh):  out = [others, X*] @ W.T + b

Host does the (tiny, ~1MB) DWT/interp exactly as the reference, then
packs a 364-feature tensor x2 = [Xl, Xh, ch1..ch361, 1.0] per token
(the ones column folds the bias in) in **fp8 e3m4** feature-major
layout, so the device work is a pure double GEMM
    out[t, 0:128]   = x2[t] @ Wg2.T
    out[t, 128:256] = x2[t] @ Wh2.T
sharded over 8 NeuronCores by stock (32 stocks/core, 32768 tokens/core).

Precision: acts fp8 e3m4 (4 mantissa bits), weights bf16, PSUM fp32,
out bf16. The PE accepts mixed bf16-stationary x fp8e3-moving matmuls
(verified on HW, rel err ~6e-8 vs the same quantized operands in f64).
End-to-end absmax_rel vs the fp32 reference is ~1.3e-2 (gate 2e-2):
the e3m4 act quantization averages out over the K=364 dot product.

Device kernel (per core):
  - input DRAM layout [slab, 364, SLAB] fp8: one 4KB/partition-line DMA
    per 128-row chunk (rows split 128/128/108), 3 DMAs per 4096-token
    slab. 11.9 MB/core vs 25.2 MB for the old bf16 layout.
  - per slab, 2 output halves x 2 PSUM waves x (3 K-chunks x 4 groups)
    of [<=128x128] x [<=128x512] matmuls accumulate into PSUM banks;
    ScalarE/VectorE copy+cast fp32 PSUM -> bf16 SBUF.
  - output DRAM layout [slab, 128, 2, SLAB] (d-major, bf16): one
    contiguous DMA per slab with 16KB/partition descriptors. The host
    de-transposes/casts while assembling the final fp32 arrays.

Roofline: 11.9 MB in + 16.8 MB out per core at ~358 GB/s ~= 80 us;
PE 384 MMs x 512 moving cols ~= 82 us warm -> ridge-balanced ~85 us,
down from the bf16 baseline's 120 us (which was HBM-bound).
"""

import os
import numpy as np
import ml_dtypes

import concourse.bacc as bacc
import concourse.mybir as mybir
import concourse.tile as tile
from concourse.bass_utils import run_bass_kernel_spmd

B, S, N, F = 2, 512, 256, 362
D = 128
NCORES = 8
NSH = N // NCORES          # 32 stocks per core
T = B * S * NSH            # 32768 tokens per core
K = F + 2                  # Xl, Xh, ch1..ch361, ones  -> 364
CHUNK_OFF = (0, 128, 256)  # non-overlapping K chunks
CHUNK_ROWS = (128, 128, K - 256)   # 128/128/108
GROUP = 512                # matmul moving-dim granularity (PSUM bank = 512 fp32)
SLAB = 4096                # tokens per DMA slab
NSLABS = T // SLAB         # 8
QPS = SLAB // GROUP        # groups per slab = 8
QBLK = 4                   # PSUM banks per accumulation wave (c-outer within)

BF16 = mybir.dt.bfloat16
F32 = mybir.dt.float32
E3 = mybir.dt.float8e3
E3_NP = ml_dtypes.float8_e3m4
OUT_BF16 = os.environ.get("KRN_OUT_F32", "0") != "1"
OUT_DT = BF16 if OUT_BF16 else F32
OUT_NP = ml_dtypes.bfloat16 if OUT_BF16 else np.float32

_NC_CACHE = {}
TRACE = False
LAST_RESULT = None


def _build(repeat=1):
    key = (OUT_BF16, repeat)
    if key in _NC_CACHE:
        return _NC_CACHE[key]
    nc = bacc.Bacc(None, target_bir_lowering=False)
    x2d = nc.dram_tensor("x2", [NSLABS, K, SLAB], E3, kind="ExternalInput")
    w2d = nc.dram_tensor("w2", [3, 128, 256], BF16, kind="ExternalInput")
    outd = nc.dram_tensor("out", [NSLABS, 128, 2, SLAB], OUT_DT, kind="ExternalOutput")

    with tile.TileContext(nc) as tc:
        with (
            tc.tile_pool(name="cpool", bufs=1) as cpool,
            tc.tile_pool(name="xpool", bufs=4) as xpool,
            tc.tile_pool(name="spool", bufs=4) as spool,
            tc.tile_pool(name="psA", bufs=8, space="PSUM") as psA,
        ):
            wt = cpool.tile([128, 3, 256], BF16)
            nc.sync.dma_start(wt[:, :, :], w2d[:, :, :].rearrange("c p d -> p c d"))

            for rep in range(repeat):
                for s in range(NSLABS):
                    xt = xpool.tile([128, 3, SLAB], E3, tag="xt")
                    for c in range(3):
                        nc.sync.dma_start(
                            xt[0 : CHUNK_ROWS[c], c, :],
                            x2d[s, CHUNK_OFF[c] : CHUNK_OFF[c] + CHUNK_ROWS[c], :],
                        )
                    so = spool.tile([128, 2, SLAB], OUT_DT, tag="so")
                    for h in range(2):
                        for qb in range(QPS // QBLK):
                            accs = [
                                psA.tile([128, GROUP], F32, tag="acc",
                                         name=f"acc{rep}_{s}_{h}_{qb}_{i}")
                                for i in range(QBLK)
                            ]
                            # c outer / q inner: one LDWEIGHTS per c feeds
                            # QBLK moving streams
                            for c in range(3):
                                r = CHUNK_ROWS[c]
                                for i in range(QBLK):
                                    q = qb * QBLK + i
                                    nc.tensor.matmul(
                                        accs[i][:, :],
                                        wt[0:r, c, h * 128 : (h + 1) * 128],
                                        xt[0:r, c, q * GROUP : (q + 1) * GROUP],
                                        start=(c == 0),
                                        stop=(c == 2),
                                    )
                            for i in range(QBLK):
                                q = qb * QBLK + i
                                dst = so[:, h, q * GROUP : (q + 1) * GROUP]
                                if (h + i) % 2 == 0:
                                    nc.scalar.copy(dst, accs[i][:, :])
                                else:
                                    nc.vector.tensor_copy(dst, accs[i][:, :])
                    nc.scalar.dma_start(outd[s, :, :, :], so[:, :, :])
    nc.finalize()
    _NC_CACHE[key] = nc
    return nc


def _haar_interp_host(x):
    """Exact fp32 replica of the reference DWT+interp, on [B, S, N] ch0."""
    r = np.ascontiguousarray(np.transpose(x[:, :, :, 0], (0, 2, 1)))  # [B, N, S]
    inv = np.float32(1.0 / np.sqrt(2.0))
    pairs = r.reshape(B, N, S // 2, 2)
    cA = (pairs[..., 0] + pairs[..., 1]) * inv
    cD = (pairs[..., 0] - pairs[..., 1]) * inv
    L = S // 2
    src = np.maximum((np.arange(S, dtype=np.float32) + 0.5) * (L / S) - 0.5, 0.0)
    i0 = np.floor(src).astype(np.int32)
    i1 = np.minimum(i0 + 1, L - 1)
    w = (src - i0.astype(np.float32)).astype(np.float32)

    def interp(c):
        return c[..., i0] * (np.float32(1.0) - w) + c[..., i1] * w  # [B, N, S]

    Xl = np.transpose(interp(cA), (0, 2, 1))  # [B, S, N]
    Xh = np.transpose(interp(cD), (0, 2, 1))
    return Xl, Xh


def _build_w2(Wg_w, Wg_b, Wh_w, Wh_b):
    W2T = np.zeros((K, 256), dtype=np.float32)
    W2T[0, :128] = Wg_w[:, F - 1]
    W2T[1, 128:] = Wh_w[:, F - 1]
    W2T[2 : F + 1, :128] = Wg_w[:, : F - 1].T
    W2T[2 : F + 1, 128:] = Wh_w[:, : F - 1].T
    W2T[F + 1, :128] = Wg_b
    W2T[F + 1, 128:] = Wh_b
    w2 = np.zeros((3, 128, 256), dtype=np.float32)
    for c in range(3):
        w2[c, 0 : CHUNK_ROWS[c]] = W2T[CHUNK_OFF[c] : CHUNK_OFF[c] + CHUNK_ROWS[c]]
    return np.ascontiguousarray(w2.astype(ml_dtypes.bfloat16))


def _core_input(x, Xl, Xh, core):
    """Build the feature-major fp8 slab layout [NSLABS, K, SLAB]."""
    n0 = core * NSH
    xa = np.ascontiguousarray(x[:, :, n0 : n0 + NSH, 1:]).reshape(T, F - 1)
    full = np.empty((K, T), dtype=E3_NP)
    full[2 : F + 1, :] = xa.T
    full[0, :] = Xl[:, :, n0 : n0 + NSH].reshape(T)
    full[1, :] = Xh[:, :, n0 : n0 + NSH].reshape(T)
    full[F + 1, :] = 1.0
    return np.ascontiguousarray(
        full.reshape(K, NSLABS, SLAB).swapaxes(0, 1)
    )


def kernel(x, Wg_w, Wg_b, Wh_w, Wh_b):
    global LAST_RESULT
    x = np.asarray(x, dtype=np.float32)
    Xl, Xh = _haar_interp_host(x)
    w2 = _build_w2(
        np.asarray(Wg_w, np.float32), np.asarray(Wg_b, np.float32),
        np.asarray(Wh_w, np.float32), np.asarray(Wh_b, np.float32),
    )

    from concurrent.futures import ThreadPoolExecutor
    with ThreadPoolExecutor(max_workers=8) as ex:
        shards = list(ex.map(lambda c: _core_input(x, Xl, Xh, c), range(NCORES)))
    in_maps = [{"x2": sh, "w2": w2} for sh in shards]

    nc = _build()
    res = run_bass_kernel_spmd(nc, in_maps, core_ids=list(range(NCORES)), trace=TRACE)
    LAST_RESULT = res

    Xl_proj = np.empty((B, S, N, D), dtype=np.float32)
    Xh_proj = np.empty((B, S, N, D), dtype=np.float32)
    for c in range(NCORES):
        o = res.results[c]["out"]  # [NSLABS, 128, 2, SLAB]
        o = np.transpose(o, (0, 3, 2, 1)).astype(np.float32)  # [NSLABS, SLAB, 2, 128]
        o = o.reshape(B, S, NSH, 2, D)
        n0 = c * NSH
        Xl_proj[:, :, n0 : n0 + NSH, :] = o[..., 0, :]
        Xh_proj[:, :, n0 : n0 + NSH, :] = o[..., 1, :]
    return Xl_proj, Xh_proj


# revision 6
# speedup vs baseline: 1.3129x; 1.0739x over previous
"""Trainium2 kernel for nn_DecouplingFlowLayer.

Reference computation (per (batch, stock) row):
  - channel 0 of x undergoes a Haar DWT + linear upsample back to S
    (low band Xl, high band Xh)
  - Xl (resp. Xh) is concatenated with channels 1..F-1 and projected by
    Wg (resp. Wh):  out = [others, X*] @ W.T + b

Host does the (tiny, ~1MB) DWT/interp exactly as the reference, then
packs a 364-feature tensor x2 = [Xl, Xh, ch1..ch361, 1.0] per token
(the ones column folds the bias in) in **fp8 e3m4** feature-major
layout, so the device work is a pure double GEMM
    out[t, 0:128]   = x2[t] @ Wg2.T
    out[t, 128:256] = x2[t] @ Wh2.T
sharded over 8 NeuronCores by stock (32 stocks/core, 32768 tokens/core).

Precision: acts fp8 e3m4 (4 mantissa bits), weights bf16, PSUM fp32,
out bf16. The PE accepts mixed bf16-stationary x fp8e3-moving matmuls
(verified on HW, rel err ~6e-8 vs the same quantized operands in f64).
End-to-end absmax_rel vs the fp32 reference is ~1.3e-2 (gate 2e-2):
the e3m4 act quantization averages out over the K=364 dot product.

Device kernel (per core):
  - input DRAM layout [slab, 364, SLAB] fp8: one 4KB/partition-line DMA
    per 128-row chunk (rows split 128/128/108), 3 DMAs per 4096-token
    slab. 11.9 MB/core vs 25.2 MB for the old bf16 layout.
  - per slab, 2 output halves x 2 PSUM waves x (3 K-chunks x 4 groups)
    of [<=128x128] x [<=128x512] matmuls accumulate into PSUM banks;
    ScalarE/VectorE copy+cast fp32 PSUM -> bf16 SBUF.
  - output DRAM layout [slab, 128, 2, SLAB] (d-major, bf16): one
    contiguous DMA per slab with 16KB/partition descriptors. The host
    de-transposes/casts while assembling the final fp32 arrays.

Roofline: 11.9 MB in + 16.8 MB out per core at ~358 GB/s ~= 80 us;
PE 384 MMs x 512 moving cols ~= 82 us warm -> ridge-balanced ~85 us,
down from the bf16 baseline's 120 us (which was HBM-bound).
"""

import os
import numpy as np
import ml_dtypes

import concourse.bacc as bacc
import concourse.mybir as mybir
import concourse.tile as tile
from concourse.bass_utils import run_bass_kernel_spmd

B, S, N, F = 2, 512, 256, 362
D = 128
NCORES = 8
NSH = N // NCORES          # 32 stocks per core
T = B * S * NSH            # 32768 tokens per core
K = F + 2                  # Xl, Xh, ch1..ch361, ones  -> 364
CHUNK_OFF = (0, 128, 256)  # non-overlapping K chunks
CHUNK_ROWS = (128, 128, K - 256)   # 128/128/108
GROUP = 512                # matmul moving-dim granularity (PSUM bank = 512 fp32)
SLAB = 2048                # tokens per DMA slab
NSLABS = T // SLAB         # 16
QPS = SLAB // GROUP        # groups per slab = 4
QBLK = 4                   # PSUM banks per accumulation wave (c-outer within)

BF16 = mybir.dt.bfloat16
F32 = mybir.dt.float32
E3 = mybir.dt.float8e3
E3_NP = ml_dtypes.float8_e3m4
OUT_BF16 = os.environ.get("KRN_OUT_F32", "0") != "1"
OUT_DT = BF16 if OUT_BF16 else F32
OUT_NP = ml_dtypes.bfloat16 if OUT_BF16 else np.float32

_NC_CACHE = {}
TRACE = False
LAST_RESULT = None


def _build(repeat=1):
    key = (OUT_BF16, repeat)
    if key in _NC_CACHE:
        return _NC_CACHE[key]
    nc = bacc.Bacc(None, target_bir_lowering=False)
    x2d = nc.dram_tensor("x2", [K, T], E3, kind="ExternalInput")
    w2d = nc.dram_tensor("w2", [3, 128, 256], BF16, kind="ExternalInput")
    outd = nc.dram_tensor("out", [128, 2, T], OUT_DT, kind="ExternalOutput")

    with tile.TileContext(nc) as tc:
        with (
            tc.tile_pool(name="cpool", bufs=1) as cpool,
            tc.tile_pool(name="xpool", bufs=4) as xpool,
            tc.tile_pool(name="spool", bufs=4) as spool,
            tc.tile_pool(name="psA", bufs=8, space="PSUM") as psA,
        ):
            wt = cpool.tile([128, 3, 256], BF16)
            nc.sync.dma_start(wt[:, :, :], w2d[:, :, :].rearrange("c p d -> p c d"))

            for rep in range(repeat):
                for s in range(NSLABS):
                    t0 = s * SLAB
                    xt = xpool.tile([128, 3, SLAB], E3, tag="xt")
                    for c in range(3):
                        nc.sync.dma_start(
                            xt[0 : CHUNK_ROWS[c], c, :],
                            x2d[CHUNK_OFF[c] : CHUNK_OFF[c] + CHUNK_ROWS[c],
                                t0 : t0 + SLAB],
                        )
                    so = spool.tile([128, 2, SLAB], OUT_DT, tag="so")
                    for h in range(2):
                        for qb in range(QPS // QBLK):
                            accs = [
                                psA.tile([128, GROUP], F32, tag="acc",
                                         name=f"acc{rep}_{s}_{h}_{qb}_{i}")
                                for i in range(QBLK)
                            ]
                            # c outer / q inner: one LDWEIGHTS per c feeds
                            # QBLK moving streams
                            for c in range(3):
                                r = CHUNK_ROWS[c]
                                for i in range(QBLK):
                                    q = qb * QBLK + i
                                    nc.tensor.matmul(
                                        accs[i][:, :],
                                        wt[0:r, c, h * 128 : (h + 1) * 128],
                                        xt[0:r, c, q * GROUP : (q + 1) * GROUP],
                                        start=(c == 0),
                                        stop=(c == 2),
                                    )
                            for i in range(QBLK):
                                q = qb * QBLK + i
                                dst = so[:, h, q * GROUP : (q + 1) * GROUP]
                                if (h + i) % 2 == 0:
                                    nc.scalar.copy(dst, accs[i][:, :])
                                else:
                                    nc.vector.tensor_copy(dst, accs[i][:, :])
                        # drain this half as soon as its copies land
                        nc.scalar.dma_start(
                            outd[:, h, t0 : t0 + SLAB], so[:, h, :]
                        )
    nc.finalize()
    _NC_CACHE[key] = nc
    return nc


def _haar_interp_host(x):
    """Exact fp32 replica of the reference DWT+interp, on [B, S, N] ch0."""
    r = np.ascontiguousarray(np.transpose(x[:, :, :, 0], (0, 2, 1)))  # [B, N, S]
    inv = np.float32(1.0 / np.sqrt(2.0))
    pairs = r.reshape(B, N, S // 2, 2)
    cA = (pairs[..., 0] + pairs[..., 1]) * inv
    cD = (pairs[..., 0] - pairs[..., 1]) * inv
    L = S // 2
    src = np.maximum((np.arange(S, dtype=np.float32) + 0.5) * (L / S) - 0.5, 0.0)
    i0 = np.floor(src).astype(np.int32)
    i1 = np.minimum(i0 + 1, L - 1)
    w = (src - i0.astype(np.float32)).astype(np.float32)

    def interp(c):
        return c[..., i0] * (np.float32(1.0) - w) + c[..., i1] * w  # [B, N, S]

    Xl = np.transpose(interp(cA), (0, 2, 1))  # [B, S, N]
    Xh = np.transpose(interp(cD), (0, 2, 1))
    return Xl, Xh


def _build_w2(Wg_w, Wg_b, Wh_w, Wh_b):
    W2T = np.zeros((K, 256), dtype=np.float32)
    W2T[0, :128] = Wg_w[:, F - 1]
    W2T[1, 128:] = Wh_w[:, F - 1]
    W2T[2 : F + 1, :128] = Wg_w[:, : F - 1].T
    W2T[2 : F + 1, 128:] = Wh_w[:, : F - 1].T
    W2T[F + 1, :128] = Wg_b
    W2T[F + 1, 128:] = Wh_b
    w2 = np.zeros((3, 128, 256), dtype=np.float32)
    for c in range(3):
        w2[c, 0 : CHUNK_ROWS[c]] = W2T[CHUNK_OFF[c] : CHUNK_OFF[c] + CHUNK_ROWS[c]]
    return np.ascontiguousarray(w2.astype(ml_dtypes.bfloat16))


def _core_input(x, Xl, Xh, core):
    """Build the feature-major fp8 layout [K, T]."""
    n0 = core * NSH
    xa = np.ascontiguousarray(x[:, :, n0 : n0 + NSH, 1:]).reshape(T, F - 1)
    full = np.empty((K, T), dtype=E3_NP)
    full[2 : F + 1, :] = xa.T
    full[0, :] = Xl[:, :, n0 : n0 + NSH].reshape(T)
    full[1, :] = Xh[:, :, n0 : n0 + NSH].reshape(T)
    full[F + 1, :] = 1.0
    return full


def kernel(x, Wg_w, Wg_b, Wh_w, Wh_b):
    global LAST_RESULT
    x = np.asarray(x, dtype=np.float32)
    Xl, Xh = _haar_interp_host(x)
    w2 = _build_w2(
        np.asarray(Wg_w, np.float32), np.asarray(Wg_b, np.float32),
        np.asarray(Wh_w, np.float32), np.asarray(Wh_b, np.float32),
    )

    from concurrent.futures import ThreadPoolExecutor
    with ThreadPoolExecutor(max_workers=8) as ex:
        shards = list(ex.map(lambda c: _core_input(x, Xl, Xh, c), range(NCORES)))
    in_maps = [{"x2": sh, "w2": w2} for sh in shards]

    nc = _build()
    res = run_bass_kernel_spmd(nc, in_maps, core_ids=list(range(NCORES)), trace=TRACE)
    LAST_RESULT = res

    Xl_proj = np.empty((B, S, N, D), dtype=np.float32)
    Xh_proj = np.empty((B, S, N, D), dtype=np.float32)
    for c in range(NCORES):
        o = res.results[c]["out"]  # [128, 2, T]
        o = np.transpose(o, (2, 1, 0)).astype(np.float32)  # [T, 2, 128]
        o = o.reshape(B, S, NSH, 2, D)
        n0 = c * NSH
        Xl_proj[:, :, n0 : n0 + NSH, :] = o[..., 0, :]
        Xh_proj[:, :, n0 : n0 + NSH, :] = o[..., 1, :]
    return Xl_proj, Xh_proj
